# revision 11
# baseline (speedup 1.0000x reference)
"""Trainium2 Bass kernel for a 4-layer GraphTransformer (PyG TransformerConv style).

Sharding: nodes are partitioned across 8 NeuronCores (graph parallel) after a
host-side load-balancing relabel; edges are partitioned by destination node so
segment-softmax/scatter are core-local. Per layer each core computes Q/K/V/skip
for its own nodes, K/V (bf16, packed per row) are all-gathered, and the edge
phase gathers per-edge K/V rows (by src) and Q rows (by dst) with indirect DMA.
Segment sums are one-hot matmuls on the TensorEngine accumulating in PSUM.
"""

import sys
import heapq
import numpy as np

sys.path.insert(0, "/opt/trn_rl_repo")

import ml_dtypes
import concourse.bass as bass
import concourse.mybir as mybir
from concourse import bacc, tile
from concourse.bass_utils import run_bass_kernel_spmd
from concourse.masks import make_identity

BF16 = ml_dtypes.bfloat16
NCORES = 8
P = 128
HEADS = 8


# --------------------------------------------------------------------------
# Host-side prep: node->slot assignment (LPT bin packing) and edge arrays
# --------------------------------------------------------------------------

def _partition_graph(n_nodes, src, dst, tiles_per_core, chunks_per_tile):
    """Assign nodes to 8*T tiles of 128 slots, balancing in-degree per tile.

    Returns slot_global[node], slot_to_node[core, slots], per-core edge arrays
    (src_slot, qdst_local, dstrel) shaped [T, 128, CH].
    """
    T = tiles_per_core
    CH = chunks_per_tile
    cap_e = CH * P
    nbins = NCORES * T
    slots_per_core = T * P

    deg = np.bincount(dst, minlength=n_nodes)
    order = np.argsort(-deg, kind="stable")

    bin_load = np.zeros(nbins, np.int64)
    bin_n = np.zeros(nbins, np.int32)
    heap = [(0, b) for b in range(nbins)]
    heapq.heapify(heap)
    node_bin = np.empty(n_nodes, np.int32)
    node_pos = np.empty(n_nodes, np.int32)
    for nd in order:
        d = int(deg[nd])
        while True:
            load, b = heapq.heappop(heap)
            if bin_n[b] < P:
                break
        node_bin[nd] = b
        node_pos[nd] = bin_n[b]
        bin_n[b] += 1
        bin_load[b] = load + d
        if bin_n[b] < P:
            heapq.heappush(heap, (load + d, b))
    if bin_load.max() > cap_e:
        raise RuntimeError(f"bin overflow: {bin_load.max()} > {cap_e}")

    # slot id: bin b covers global slots [b*128, (b+1)*128); core = b // T
    slot_global = node_bin.astype(np.int64) * P + node_pos
    slot_to_node = np.full((NCORES, slots_per_core), -1, np.int64)
    cores = node_bin // T
    local = (node_bin % T) * P + node_pos
    slot_to_node[cores, local] = np.arange(n_nodes)

    # edges sorted by dst slot -> grouped by (core, tile)
    dslot = slot_global[dst]
    order_e = np.argsort(dslot, kind="stable")
    s_sorted = slot_global[src][order_e]
    d_sorted = dslot[order_e]
    ebin = (d_sorted // P).astype(np.int64)
    counts = np.bincount(ebin, minlength=nbins)
    assert counts.max() <= cap_e
    offs = np.zeros(nbins + 1, np.int64)
    np.cumsum(counts, out=offs[1:])
    pos = np.arange(len(d_sorted)) - offs[ebin]

    src_p = np.zeros((nbins, cap_e), np.int32)
    qdst_p = np.zeros((nbins, cap_e), np.int32)
    drel_p = np.full((nbins, cap_e), -1.0, np.float32)
    src_p[ebin, pos] = s_sorted
    qdst_p[ebin, pos] = d_sorted % slots_per_core  # local slot within core
    drel_p[ebin, pos] = d_sorted % P

    def per_core(arr, dt):
        # [nbins, cap_e] -> per core [T, 128, CH]   (edge j: chunk=j//128, p=j%128)
        a = arr.reshape(NCORES, T, CH, P).transpose(0, 1, 3, 2)
        return [np.ascontiguousarray(a[c]).astype(dt) for c in range(NCORES)]

    return (
        slot_global,
        slot_to_node,
        per_core(src_p, np.int32),
        per_core(drel_p, np.float32),
    )


# --------------------------------------------------------------------------
# Device program builder
# --------------------------------------------------------------------------

def _build_program(gene, layers, T, CH, nclasses):
    """layers: list of (fin, fout) for the 4 convs."""
    SLOTS = T * P
    NCH = SLOTS // 512  # node-phase column chunks

    nc = bacc.Bacc("TRN2", target_bir_lowering=False, debug=False,
                   num_devices=NCORES)
    f32, bf, i32 = mybir.dt.float32, mybir.dt.bfloat16, mybir.dt.int32

    xT = nc.dram_tensor("xT", [gene, SLOTS], bf, kind="ExternalInput")
    srcI = nc.dram_tensor("srci", [T, P, CH], i32, kind="ExternalInput")
    drelI = nc.dram_tensor("drel", [T, P, CH], f32, kind="ExternalInput")
    Ws, Bs, ABs = [], [], []
    for li, (fin, fout) in enumerate(layers):
        Ws.append(nc.dram_tensor(f"w{li}", [fin, 4 * fout], bf, kind="ExternalInput"))
        Bs.append(nc.dram_tensor(f"b{li}", [fout, 4], f32, kind="ExternalInput"))
        ABs.append(nc.dram_tensor(f"ab{li}", [fout, 2], f32, kind="ExternalInput"))
    Wc = nc.dram_tensor("wc", [layers[-1][1], nclasses], f32, kind="ExternalInput")
    Bc = nc.dram_tensor("bc", [nclasses, 1], f32, kind="ExternalInput")

    h4T = nc.dram_tensor("h4T", [layers[-1][1], SLOTS], f32, kind="ExternalOutput")
    hbn4T = nc.dram_tensor("hbn4T", [layers[-1][1], SLOTS], f32, kind="ExternalOutput")
    lgT = nc.dram_tensor("lgT", [nclasses, SLOTS], f32, kind="ExternalOutput")

    with tile.TileContext(nc) as tc:
        with (
            tc.tile_pool(name="const", bufs=1) as cpool,
            tc.tile_pool(name="hres", bufs=2) as hpool,
            tc.tile_pool(name="work", bufs=3) as sb,
            tc.tile_pool(name="nwork", bufs=3) as nsb,
            tc.tile_pool(name="psA", bufs=2, space="PSUM") as psA,
            tc.tile_pool(name="psB", bufs=2, space="PSUM") as psB,
            tc.tile_pool(name="psC", bufs=2, space="PSUM") as psC,
            tc.tile_pool(name="dram", bufs=1, space="DRAM") as dr,
        ):
            ident = cpool.tile([P, P], f32, name="ident")
            make_identity(nc, ident[:])
            iota_i = cpool.tile([P, P], i32, name="iota_i")
            nc.gpsimd.iota(iota_i[:], pattern=[[1, P]], base=0, channel_multiplier=0)
            iota_r = cpool.tile([P, P], f32, name="iota_r")  # [p, j] = j
            nc.vector.tensor_copy(out=iota_r[:], in_=iota_i[:])
            iota_ci = cpool.tile([P, P], i32, name="iota_ci")
            nc.gpsimd.iota(iota_ci[:], pattern=[[0, P]], base=0, channel_multiplier=1)
            iota_c = cpool.tile([P, P], f32, name="iota_c")  # [p, j] = p
            nc.vector.tensor_copy(out=iota_c[:], in_=iota_ci[:])

            # per-layer weights resident in SBUF
            wsb, bsb_t, absb = [], [], []
            for li, (fin, fout) in enumerate(layers):
                kt = fin // P
                w = cpool.tile([P, kt * 4 * fout], bf, name=f"wsb{li}")
                for k in range(kt):
                    nc.sync.dma_start(
                        out=w[:, k * 4 * fout:(k + 1) * 4 * fout],
                        in_=Ws[li].ap()[k * P:(k + 1) * P, :],
                    )
                b = cpool.tile([P, 4], f32, name=f"bsb{li}")
                nc.sync.dma_start(out=b[:fout], in_=Bs[li].ap()[:, :])
                ab = cpool.tile([P, 2], f32, name=f"absb{li}")
                nc.sync.dma_start(out=ab[:fout], in_=ABs[li].ap()[:, :])
                wsb.append(w); bsb_t.append(b); absb.append(ab)
            wc_sb = cpool.tile([P, nclasses], f32, name="wcsb")
            nc.sync.dma_start(out=wc_sb[: layers[-1][1]], in_=Wc.ap()[:, :])
            bc_sb = cpool.tile([P, 1], f32, name="bcsb")
            nc.sync.dma_start(out=bc_sb[:nclasses], in_=Bc.ap()[:, :])

            h_cur = None
            for li, (fin, fout) in enumerate(layers):
                kt = fin // P
                d = fout // HEADS
                last = li == len(layers) - 1
                q_dram = dr.tile([SLOTS, fout], bf, name=f"qd{li}")
                kv_core = dr.tile([SLOTS, 2 * fout], bf, name=f"kvc{li}")
                kv_full = dr.tile([NCORES * SLOTS, 2 * fout], bf,
                                  name=f"kvf{li}", addr_space="Shared")
                sT_dram = dr.tile([fout, SLOTS], f32, name=f"sd{li}")
                h_next = None if last else hpool.tile([P, SLOTS], bf, name=f"h{li}",
                                                      tag="hres")

                # ---------------- node phase ----------------
                for ncc in range(NCH):
                    c0 = ncc * 512
                    if li == 0:
                        xt = nsb.tile([P, kt * 512], bf, name="xt")
                        for k in range(kt):
                            nc.sync.dma_start(
                                out=xt[:, k * 512:(k + 1) * 512],
                                in_=xT.ap()[k * P:(k + 1) * P, c0:c0 + 512],
                            )
                        rhs_t = [xt[:, k * 512:(k + 1) * 512] for k in range(kt)]
                    else:
                        rhs_t = [h_cur[:, c0:c0 + 512]]

                    for j in range(4):  # q, k, v, s
                        ps = psA.tile([P, 512], f32, name="psproj", tag="psproj")
                        for k in range(kt):
                            nc.tensor.matmul(
                                ps[:fout],
                                lhsT=wsb[li][:, (k * 4 + j) * fout:(k * 4 + j + 1) * fout],
                                rhs=rhs_t[k],
                                start=(k == 0),
                                stop=(k == kt - 1),
                            )
                        fs = nsb.tile([P, 512], f32, name="fs", tag="fs")
                        nc.vector.tensor_scalar(
                            out=fs[:fout], in0=ps[:fout],
                            scalar1=bsb_t[li][:fout, j:j + 1], scalar2=None,
                            op0=mybir.AluOpType.add,
                        )
                        if j == 3:  # skip proj: store transposed f32
                            nc.sync.dma_start(
                                out=sT_dram[:, c0:c0 + 512], in_=fs[:fout])
                            continue
                        bb = nsb.tile([P, 4 * fout], bf, name="bb", tag=f"bb{j}")
                        for i in range(4):
                            pt = psB.tile([P, P], f32, name="pt", tag="pt")
                            nc.tensor.transpose(
                                out=pt[:, :fout],
                                in_=fs[:fout, i * P:(i + 1) * P],
                                identity=ident[:fout, :fout],
                            )
                            nc.scalar.copy(
                                out=bb[:, i * fout:(i + 1) * fout],
                                in_=pt[:, :fout])
                        if j == 0:
                            oap = q_dram[c0:c0 + 512, :]
                        elif j == 1:
                            oap = kv_core[c0:c0 + 512, 0:fout]
                        else:
                            oap = kv_core[c0:c0 + 512, fout:2 * fout]
                        nc.sync.dma_start(
                            out=oap.rearrange("(i p) f -> p i f", p=P),
                            in_=bb[:].rearrange("p (i f) -> p i f", i=4),
                        )

                # ---------------- all-gather K/V ----------------
                nc.gpsimd.collective_compute(
                    "AllGather", mybir.AluOpType.bypass,
                    replica_groups=[list(range(NCORES))],
                    ins=[kv_core.opt()], outs=[kv_full.opt()],
                )

                # ---------------- edge phase ----------------
                F2 = 2 * fout
                FR = fout + HEADS  # rhs cols per chunk: [wv | exp]
                for t in range(T):
                    src_s = sb.tile([P, CH], i32, name="srcs", tag="srcs")
                    nc.sync.dma_start(out=src_s[:], in_=srcI.ap()[t])
                    dr_s = sb.tile([P, CH], f32, name="drs", tag="drs")
                    nc.sync.dma_start(out=dr_s[:], in_=drelI.ap()[t])
                    q_t = sb.tile([P, fout], bf, name="q_t", tag="q_t")
                    nc.sync.dma_start(
                        out=q_t[:], in_=q_dram[t * P:(t + 1) * P, :])

                    kvt = sb.tile([P, CH * F2], bf, name="kvt", tag="kvt")
                    for c in range(CH):
                        nc.gpsimd.indirect_dma_start(
                            out=kvt[:, c * F2:(c + 1) * F2], out_offset=None,
                            in_=kv_full[:],
                            in_offset=bass.IndirectOffsetOnAxis(
                                ap=src_s[:, c:c + 1], axis=0),
                        )

                    # expand q to edges: per chunk build S[n, e] one-hot and
                    # matmul with the dense q tile of this dst window
                    qt = sb.tile([P, CH * fout], bf, name="qt", tag="qt")
                    for c in range(CH):
                        ptt = psB.tile([P, P], f32, name="ptt", tag="pt")
                        nc.tensor.transpose(
                            out=ptt[:],
                            in_=dr_s[:, c:c + 1].to_broadcast([P, P]),
                            identity=ident[:])
                        tts = sb.tile([P, P], f32, name="tts", tag="tts")
                        nc.scalar.copy(out=tts[:], in_=ptt[:])
                        S = sb.tile([P, P], bf, name="S", tag="S")
                        nc.vector.tensor_tensor(
                            out=S[:], in0=tts[:], in1=iota_c[:],
                            op=mybir.AluOpType.is_equal)
                        qe_ps = psB.tile([P, P], f32, name="qe_ps", tag="pt")
                        nc.tensor.matmul(
                            qe_ps[:, :fout], lhsT=S[:], rhs=q_t[:],
                            start=True, stop=True)
                        nc.scalar.copy(
                            out=qt[:, c * fout:(c + 1) * fout],
                            in_=qe_ps[:, :fout])

                    kv3 = kvt[:].rearrange("p (c f) -> p c f", c=CH)
                    qk = sb.tile([P, CH * fout], f32, name="qk", tag="qk")
                    nc.vector.tensor_tensor(
                        out=qk[:].rearrange("p (c f) -> p c f", c=CH),
                        in0=qt[:].rearrange("p (c f) -> p c f", c=CH),
                        in1=kv3[:, :, 0:fout],
                        op=mybir.AluOpType.mult,
                    )
                    al = sb.tile([P, CH * HEADS], f32, name="al", tag="al")
                    nc.vector.tensor_reduce(
                        out=al[:].rearrange("p (x o) -> p x o", o=1),
                        in_=qk[:].rearrange("p (x dd) -> p x dd", dd=d),
                        axis=mybir.AxisListType.X,
                        op=mybir.AluOpType.add,
                    )
                    rhs_all = sb.tile([P, CH * FR], bf, name="rhsall", tag="rhsall")
                    r3 = rhs_all[:].rearrange("p (c f) -> p c f", c=CH)
                    nc.scalar.activation(
                        out=r3[:, :, fout:FR],
                        in_=al[:].rearrange("p (c h) -> p c h", c=CH),
                        func=mybir.ActivationFunctionType.Exp,
                        scale=float(1.0 / np.sqrt(d)),
                    )
                    nc.vector.tensor_tensor(
                        out=r3[:, :, 0:fout].rearrange("p c (h dd) -> p c h dd", h=HEADS),
                        in0=r3[:, :, fout:FR][:, :, :, None].to_broadcast(
                            [P, CH, HEADS, d]),
                        in1=kv3[:, :, fout:F2].rearrange(
                            "p c (h dd) -> p c h dd", h=HEADS),
                        op=mybir.AluOpType.mult,
                    )
                    st = sb.tile([P, CH * P], bf, name="st", tag="st")
                    nc.vector.tensor_tensor(
                        out=st[:].rearrange("p (c n) -> p c n", c=CH),
                        in0=dr_s[:, :, None].to_broadcast([P, CH, P]),
                        in1=iota_r[:, None, :].to_broadcast([P, CH, P]),
                        op=mybir.AluOpType.is_equal,
                    )
                    pa = psC.tile([P, FR], f32, name="pa", tag="pa")
                    for c in range(CH):
                        nc.tensor.matmul(
                            pa[:],
                            lhsT=st[:, c * P:(c + 1) * P],
                            rhs=rhs_all[:, c * FR:(c + 1) * FR],
                            start=(c == 0),
                            stop=(c == CH - 1),
                        )
                    den = sb.tile([P, HEADS], f32, name="den", tag="den")
                    nc.vector.tensor_scalar(
                        out=den[:], in0=pa[:, fout:FR], scalar1=1e-16,
                        scalar2=None, op0=mybir.AluOpType.add)
                    rec = sb.tile([P, HEADS], f32, name="rec", tag="rec")
                    nc.vector.reciprocal(out=rec[:], in_=den[:])
                    y = sb.tile([P, fout], f32, name="y", tag="y")
                    nc.vector.tensor_tensor(
                        out=y[:].rearrange("p (h dd) -> p h dd", h=HEADS),
                        in0=pa[:, 0:fout].rearrange("p (h dd) -> p h dd", h=HEADS),
                        in1=rec[:, :, None].to_broadcast([P, HEADS, d]),
                        op=mybir.AluOpType.mult,
                    )
                    yt = psB.tile([P, P], f32, name="yt", tag="pt")
                    nc.tensor.transpose(out=yt[:fout, :], in_=y[:], identity=ident[:])
                    s_s = sb.tile([P, P], f32, name="ss", tag="ss")
                    nc.sync.dma_start(
                        out=s_s[:fout], in_=sT_dram[:, t * P:(t + 1) * P])
                    t1 = sb.tile([P, P], f32, name="t1", tag="t1")
                    nc.vector.tensor_tensor(
                        out=t1[:fout], in0=yt[:fout, :], in1=s_s[:fout],
                        op=mybir.AluOpType.add)
                    if not last:
                        t2 = sb.tile([P, P], f32, name="t2", tag="t2")
                        nc.vector.tensor_scalar(
                            out=t2[:fout], in0=t1[:fout], scalar1=0.0,
                            scalar2=None, op0=mybir.AluOpType.max)
                        nc.vector.tensor_scalar(
                            out=h_next[:, t * P:(t + 1) * P], in0=t2[:fout],
                            scalar1=absb[li][:fout, 0:1],
                            scalar2=absb[li][:fout, 1:2],
                            op0=mybir.AluOpType.mult, op1=mybir.AluOpType.add)
                    else:
                        h4s = sb.tile([P, P], f32, name="h4s", tag="t2")
                        nc.vector.tensor_scalar(
                            out=h4s[:fout], in0=t1[:fout], scalar1=0.0,
                            scalar2=None, op0=mybir.AluOpType.max)
                        nc.sync.dma_start(
                            out=h4T.ap()[:, t * P:(t + 1) * P], in_=h4s[:fout])
                        hbn = sb.tile([P, P], f32, name="hbn", tag="hbn")
                        nc.vector.tensor_scalar(
                            out=hbn[:fout], in0=h4s[:fout],
                            scalar1=absb[li][:fout, 0:1],
                            scalar2=absb[li][:fout, 1:2],
                            op0=mybir.AluOpType.mult, op1=mybir.AluOpType.add)
                        nc.sync.dma_start(
                            out=hbn4T.ap()[:, t * P:(t + 1) * P], in_=hbn[:fout])
                        pl = psB.tile([P, P], f32, name="pl", tag="pt")
                        nc.tensor.matmul(
                            pl[:nclasses, :P], lhsT=wc_sb[:fout],
                            rhs=hbn[:fout], start=True, stop=True)
                        lg = sb.tile([P, P], f32, name="lg", tag="lg")
                        nc.vector.tensor_scalar(
                            out=lg[:nclasses], in0=pl[:nclasses, :P],
                            scalar1=bc_sb[:nclasses, 0:1], scalar2=None,
                            op0=mybir.AluOpType.add)
                        nc.sync.dma_start(
                            out=lgT.ap()[:, t * P:(t + 1) * P],
                            in_=lg[:nclasses])
                h_cur = h_next

    nc.compile()
    return nc


# --------------------------------------------------------------------------
# Public entry point
# --------------------------------------------------------------------------

_CACHE = {}
_LAST_EXEC_NS = None


def kernel_timed(x, edge_index, params):
    """Run kernel with HW profiling; returns max per-core exec_time_ns."""
    global _TRACE
    _TRACE = True
    try:
        kernel(x, edge_index, params)
    finally:
        _TRACE = False
    return _LAST_EXEC_NS


_TRACE = False


def kernel(x, edge_index, params):
    x = np.asarray(x, np.float32)
    edge_index = np.asarray(edge_index)
    n_nodes, gene = x.shape
    src, dst = np.asarray(edge_index[0], np.int64), np.asarray(edge_index[1], np.int64)

    layers = []
    for nm in ("conv1", "conv2", "conv3", "conv4"):
        w = np.asarray(params[nm]["Wq"])
        layers.append((w.shape[0], w.shape[1]))
    nclasses = np.asarray(params["Wc"]).shape[1]

    # tiles per core: smallest T with ~4% headroom for LPT balance
    n_edges = len(src)
    CH = 8
    T = int(np.ceil(n_edges / NCORES * 1.04 / (CH * P)))
    T = max(T, int(np.ceil(n_nodes / (NCORES * P))))
    SLOTS = T * P
    while (SLOTS % 512) != 0:
        T += 1
        SLOTS = T * P

    (slot_global, slot_to_node, src_pc, drel_pc) = _partition_graph(
        n_nodes, src, dst, T, CH)

    key = (n_nodes, gene, tuple(layers), T, CH, nclasses)
    if key not in _CACHE:
        _CACHE[key] = _build_program(gene, layers, T, CH, nclasses)
    nc = _CACHE[key]

    # per-core inputs
    xbf = x.astype(BF16)
    BN_EPS = 1e-5
    wl, bl, abl = [], [], []
    for li, nm in enumerate(("conv1", "conv2", "conv3", "conv4")):
        p = params[nm]
        W = np.stack([np.asarray(p[k], np.float32) for k in ("Wq", "Wk", "Wv", "Ws")], 1)
        fin, _, fout = W.shape
        wl.append(np.ascontiguousarray(W.reshape(fin, 4 * fout)).astype(BF16))
        bl.append(np.ascontiguousarray(np.stack(
            [np.asarray(p[k], np.float32) for k in ("bq", "bk", "bv", "bs")], 1)))
        bn = params[f"bn{li + 1}"]
        A = np.asarray(bn["g"], np.float32) / np.sqrt(np.asarray(bn["v"], np.float32) + BN_EPS)
        B = np.asarray(bn["b"], np.float32) - np.asarray(bn["m"], np.float32) * A
        abl.append(np.ascontiguousarray(np.stack([A, B], 1)))
    wc = np.asarray(params["Wc"], np.float32)
    bc = np.asarray(params["bc"], np.float32).reshape(-1, 1)

    in_maps = []
    for c in range(NCORES):
        s2n = slot_to_node[c]
        valid = s2n >= 0
        xt = np.zeros((gene, SLOTS), BF16)
        xt[:, valid] = xbf[s2n[valid]].T
        m = {
            "xT": xt,
            "srci": src_pc[c], "drel": drel_pc[c],
            "wc": wc, "bc": bc,
        }
        for li in range(4):
            m[f"w{li}"] = wl[li]
            m[f"b{li}"] = bl[li]
            m[f"ab{li}"] = abl[li]
        in_maps.append(m)

    res = run_bass_kernel_spmd(nc, in_maps, core_ids=list(range(NCORES)),
                               trace=_TRACE)
    global _LAST_EXEC_NS
    if _TRACE:
        _LAST_EXEC_NS = res.exec_time_ns

    fout4 = layers[-1][1]
    logits = np.zeros((n_nodes, nclasses), np.float32)
    h4 = np.zeros((n_nodes, fout4), np.float32)
    hbn4 = np.zeros((n_nodes, fout4), np.float32)
    for c in range(NCORES):
        s2n = slot_to_node[c]
        valid = s2n >= 0
        nd = s2n[valid]
        logits[nd] = res.results[c]["lgT"][:, valid].T
        h4[nd] = res.results[c]["h4T"][:, valid].T
        hbn4[nd] = res.results[c]["hbn4T"][:, valid].T
    return logits, h4, hbn4


# revision 12
# speedup vs baseline: 1.4062x; 1.4062x over previous
"""Trainium2 Bass kernel for a 4-layer GraphTransformer (PyG TransformerConv style).

Sharding: nodes are partitioned across 8 NeuronCores (graph parallel) after a
host-side load-balancing relabel; edges are partitioned by destination node so
segment-softmax/scatter are core-local. Per layer each core computes Q/K/V/skip
for its own nodes, K/V (bf16, packed per row) are all-gathered, and the edge
phase gathers per-edge K/V rows (by src) and Q rows (by dst) with indirect DMA.
Segment sums are one-hot matmuls on the TensorEngine accumulating in PSUM.
"""

import sys
import heapq
import numpy as np

sys.path.insert(0, "/opt/trn_rl_repo")

import ml_dtypes
import concourse.bass as bass
import concourse.mybir as mybir
from concourse import bacc, tile
from concourse.bass_utils import run_bass_kernel_spmd
from concourse.masks import make_identity

BF16 = ml_dtypes.bfloat16
NCORES = 8
P = 128
HEADS = 8


# --------------------------------------------------------------------------
# Host-side prep: node->slot assignment (LPT bin packing) and edge arrays
# --------------------------------------------------------------------------

def _partition_graph(n_nodes, src, dst, tiles_per_core, chunks_per_tile):
    """Assign nodes to 8*T tiles of 128 slots, balancing in-degree per tile.

    Returns slot_global[node], slot_to_node[core, slots], per-core edge arrays
    (src_slot, qdst_local, dstrel) shaped [T, 128, CH].
    """
    T = tiles_per_core
    CH = chunks_per_tile
    cap_e = CH * P
    nbins = NCORES * T
    slots_per_core = T * P

    deg = np.bincount(dst, minlength=n_nodes)
    order = np.argsort(-deg, kind="stable")

    bin_load = np.zeros(nbins, np.int64)
    bin_n = np.zeros(nbins, np.int32)
    heap = [(0, b) for b in range(nbins)]
    heapq.heapify(heap)
    node_bin = np.empty(n_nodes, np.int32)
    node_pos = np.empty(n_nodes, np.int32)
    for nd in order:
        d = int(deg[nd])
        while True:
            load, b = heapq.heappop(heap)
            if bin_n[b] < P:
                break
        node_bin[nd] = b
        node_pos[nd] = bin_n[b]
        bin_n[b] += 1
        bin_load[b] = load + d
        if bin_n[b] < P:
            heapq.heappush(heap, (load + d, b))
    if bin_load.max() > cap_e:
        raise RuntimeError(f"bin overflow: {bin_load.max()} > {cap_e}")

    # slot id: bin b covers global slots [b*128, (b+1)*128); core = b // T
    slot_global = node_bin.astype(np.int64) * P + node_pos
    slot_to_node = np.full((NCORES, slots_per_core), -1, np.int64)
    cores = node_bin // T
    local = (node_bin % T) * P + node_pos
    slot_to_node[cores, local] = np.arange(n_nodes)

    # edges sorted by dst slot -> grouped by (core, tile)
    dslot = slot_global[dst]
    order_e = np.argsort(dslot, kind="stable")
    s_sorted = slot_global[src][order_e]
    d_sorted = dslot[order_e]
    ebin = (d_sorted // P).astype(np.int64)
    counts = np.bincount(ebin, minlength=nbins)
    assert counts.max() <= cap_e
    offs = np.zeros(nbins + 1, np.int64)
    np.cumsum(counts, out=offs[1:])
    pos = np.arange(len(d_sorted)) - offs[ebin]

    src_p = np.zeros((nbins, cap_e), np.int32)
    qdst_p = np.zeros((nbins, cap_e), np.int32)
    drel_p = np.full((nbins, cap_e), -1.0, np.float32)
    src_p[ebin, pos] = s_sorted
    qdst_p[ebin, pos] = d_sorted % slots_per_core  # local slot within core
    drel_p[ebin, pos] = d_sorted % P

    def per_core(arr, dt):
        # [nbins, cap_e] -> per core [T, 128, CH]   (edge j: chunk=j//128, p=j%128)
        a = arr.reshape(NCORES, T, CH, P).transpose(0, 1, 3, 2)
        return [np.ascontiguousarray(a[c]).astype(dt) for c in range(NCORES)]

    return (
        slot_global,
        slot_to_node,
        per_core(src_p, np.int32),
        per_core(drel_p, np.float32),
    )


# --------------------------------------------------------------------------
# Device program builder
# --------------------------------------------------------------------------

def _build_program(gene, layers, T, CH, nclasses):
    """layers: list of (fin, fout) for the 4 convs."""
    SLOTS = T * P
    NCH = SLOTS // 512  # node-phase column chunks

    nc = bacc.Bacc("TRN2", target_bir_lowering=False, debug=False,
                   num_devices=NCORES)
    f32, bf, i32 = mybir.dt.float32, mybir.dt.bfloat16, mybir.dt.int32

    xT = nc.dram_tensor("xT", [gene, SLOTS], bf, kind="ExternalInput")
    srcI = nc.dram_tensor("srci", [T, P, CH], i32, kind="ExternalInput")
    drelI = nc.dram_tensor("drel", [T, P, CH], f32, kind="ExternalInput")
    Ws, Bs, ABs = [], [], []
    for li, (fin, fout) in enumerate(layers):
        Ws.append(nc.dram_tensor(f"w{li}", [fin, 4 * fout], bf, kind="ExternalInput"))
        Bs.append(nc.dram_tensor(f"b{li}", [fout, 4], f32, kind="ExternalInput"))
        ABs.append(nc.dram_tensor(f"ab{li}", [fout, 2], f32, kind="ExternalInput"))
    Wc = nc.dram_tensor("wc", [layers[-1][1], nclasses], f32, kind="ExternalInput")
    Bc = nc.dram_tensor("bc", [nclasses, 1], f32, kind="ExternalInput")

    h4T = nc.dram_tensor("h4T", [layers[-1][1], SLOTS], f32, kind="ExternalOutput")
    hbn4T = nc.dram_tensor("hbn4T", [layers[-1][1], SLOTS], f32, kind="ExternalOutput")
    lgT = nc.dram_tensor("lgT", [nclasses, SLOTS], f32, kind="ExternalOutput")

    with tile.TileContext(nc) as tc:
        with (
            tc.tile_pool(name="const", bufs=1) as cpool,
            tc.tile_pool(name="hres", bufs=2) as hpool,
            tc.tile_pool(name="work", bufs=3) as sb,
            tc.tile_pool(name="nwork", bufs=3) as nsb,
            tc.tile_pool(name="psA", bufs=2, space="PSUM") as psA,
            tc.tile_pool(name="psB", bufs=2, space="PSUM") as psB,
            tc.tile_pool(name="psC", bufs=2, space="PSUM") as psC,
            tc.tile_pool(name="dram", bufs=1, space="DRAM") as dr,
        ):
            ident = cpool.tile([P, P], f32, name="ident")
            make_identity(nc, ident[:])
            iota_i = cpool.tile([P, P], i32, name="iota_i")
            nc.gpsimd.iota(iota_i[:], pattern=[[1, P]], base=0, channel_multiplier=0)
            iota_r = cpool.tile([P, P], f32, name="iota_r")  # [p, j] = j
            nc.vector.tensor_copy(out=iota_r[:], in_=iota_i[:])
            iota_ci = cpool.tile([P, P], i32, name="iota_ci")
            nc.gpsimd.iota(iota_ci[:], pattern=[[0, P]], base=0, channel_multiplier=1)
            iota_c = cpool.tile([P, P], f32, name="iota_c")  # [p, j] = p
            nc.vector.tensor_copy(out=iota_c[:], in_=iota_ci[:])

            # per-layer weights resident in SBUF
            wsb, bsb_t, absb = [], [], []
            for li, (fin, fout) in enumerate(layers):
                kt = fin // P
                w = cpool.tile([P, kt * 4 * fout], bf, name=f"wsb{li}")
                for k in range(kt):
                    nc.sync.dma_start(
                        out=w[:, k * 4 * fout:(k + 1) * 4 * fout],
                        in_=Ws[li].ap()[k * P:(k + 1) * P, :],
                    )
                b = cpool.tile([P, 4], f32, name=f"bsb{li}")
                nc.sync.dma_start(out=b[:fout], in_=Bs[li].ap()[:, :])
                ab = cpool.tile([P, 2], f32, name=f"absb{li}")
                nc.sync.dma_start(out=ab[:fout], in_=ABs[li].ap()[:, :])
                wsb.append(w); bsb_t.append(b); absb.append(ab)
            wc_sb = cpool.tile([P, nclasses], f32, name="wcsb")
            nc.sync.dma_start(out=wc_sb[: layers[-1][1]], in_=Wc.ap()[:, :])
            bc_sb = cpool.tile([P, 1], f32, name="bcsb")
            nc.sync.dma_start(out=bc_sb[:nclasses], in_=Bc.ap()[:, :])

            h_cur = None
            for li, (fin, fout) in enumerate(layers):
                kt = fin // P
                d = fout // HEADS
                last = li == len(layers) - 1
                q_dram = dr.tile([SLOTS, fout], bf, name=f"qd{li}")
                kv_core = dr.tile([SLOTS, 2 * fout], bf, name=f"kvc{li}")
                kv_full = dr.tile([NCORES * SLOTS, 2 * fout], bf,
                                  name=f"kvf{li}", addr_space="Shared")
                sT_dram = dr.tile([fout, SLOTS], f32, name=f"sd{li}")
                h_next = None if last else hpool.tile([P, SLOTS], bf, name=f"h{li}",
                                                      tag="hres")

                # ---------------- node phase ----------------
                for ncc in range(NCH):
                    c0 = ncc * 512
                    if li == 0:
                        xt = nsb.tile([P, kt * 512], bf, name="xt")
                        for k in range(kt):
                            nc.sync.dma_start(
                                out=xt[:, k * 512:(k + 1) * 512],
                                in_=xT.ap()[k * P:(k + 1) * P, c0:c0 + 512],
                            )
                        rhs_t = [xt[:, k * 512:(k + 1) * 512] for k in range(kt)]
                    else:
                        rhs_t = [h_cur[:, c0:c0 + 512]]

                    for j in range(4):  # q, k, v, s
                        ps = psA.tile([P, 512], f32, name="psproj", tag="psproj")
                        for k in range(kt):
                            nc.tensor.matmul(
                                ps[:fout],
                                lhsT=wsb[li][:, (k * 4 + j) * fout:(k * 4 + j + 1) * fout],
                                rhs=rhs_t[k],
                                start=(k == 0),
                                stop=(k == kt - 1),
                            )
                        fs = nsb.tile([P, 512], f32, name="fs", tag="fs")
                        nc.vector.tensor_scalar(
                            out=fs[:fout], in0=ps[:fout],
                            scalar1=bsb_t[li][:fout, j:j + 1], scalar2=None,
                            op0=mybir.AluOpType.add,
                        )
                        if j == 3:  # skip proj: store transposed f32
                            nc.sync.dma_start(
                                out=sT_dram[:, c0:c0 + 512], in_=fs[:fout])
                            continue
                        bb = nsb.tile([P, 4 * fout], bf, name="bb", tag=f"bb{j}")
                        for i in range(4):
                            pt = psB.tile([P, P], f32, name="pt", tag="pt")
                            nc.tensor.transpose(
                                out=pt[:, :fout],
                                in_=fs[:fout, i * P:(i + 1) * P],
                                identity=ident[:fout, :fout],
                            )
                            nc.scalar.copy(
                                out=bb[:, i * fout:(i + 1) * fout],
                                in_=pt[:, :fout])
                        if j == 0:
                            oap = q_dram[c0:c0 + 512, :]
                        elif j == 1:
                            oap = kv_core[c0:c0 + 512, 0:fout]
                        else:
                            oap = kv_core[c0:c0 + 512, fout:2 * fout]
                        nc.sync.dma_start(
                            out=oap.rearrange("(i p) f -> p i f", p=P),
                            in_=bb[:].rearrange("p (i f) -> p i f", i=4),
                        )

                # ---------------- all-gather K/V ----------------
                nc.gpsimd.collective_compute(
                    "AllGather", mybir.AluOpType.bypass,
                    replica_groups=[list(range(NCORES))],
                    ins=[kv_core.opt()], outs=[kv_full.opt()],
                )

                # ---------------- edge phase ----------------
                F2 = 2 * fout
                FR = fout + HEADS  # rhs cols per chunk: [wv | exp]
                for t in range(T):
                    src_s = sb.tile([P, CH], i32, name="srcs", tag="srcs")
                    nc.sync.dma_start(out=src_s[:], in_=srcI.ap()[t])
                    dr_s = sb.tile([P, CH], f32, name="drs", tag="drs")
                    nc.sync.dma_start(out=dr_s[:], in_=drelI.ap()[t])
                    q_t = sb.tile([P, fout], bf, name="q_t", tag="q_t")
                    nc.sync.dma_start(
                        out=q_t[:], in_=q_dram[t * P:(t + 1) * P, :])

                    kvt = sb.tile([P, CH * F2], bf, name="kvt", tag="kvt")
                    for c in range(CH):
                        nc.gpsimd.indirect_dma_start(
                            out=kvt[:, c * F2:(c + 1) * F2], out_offset=None,
                            in_=kv_full[:],
                            in_offset=bass.IndirectOffsetOnAxis(
                                ap=src_s[:, c:c + 1], axis=0),
                        )

                    # expand q to edges: per chunk build S[n, e] one-hot and
                    # matmul with the dense q tile of this dst window
                    qt = sb.tile([P, CH * fout], bf, name="qt", tag="qt")
                    for c in range(CH):
                        ptt = psB.tile([P, P], f32, name="ptt", tag="ptt")
                        nc.tensor.transpose(
                            out=ptt[:],
                            in_=dr_s[:, c:c + 1].to_broadcast([P, P]),
                            identity=ident[:])
                        tts = sb.tile([P, P], f32, name="tts", tag="tts")
                        nc.scalar.copy(out=tts[:], in_=ptt[:])
                        S = sb.tile([P, P], bf, name="S", tag="S")
                        nc.vector.tensor_tensor(
                            out=S[:], in0=tts[:], in1=iota_c[:],
                            op=mybir.AluOpType.is_equal)
                        qe_ps = psB.tile([P, P], f32, name="qe_ps", tag="pt")
                        nc.tensor.matmul(
                            qe_ps[:, :fout], lhsT=S[:], rhs=q_t[:],
                            start=True, stop=True)
                        nc.scalar.copy(
                            out=qt[:, c * fout:(c + 1) * fout],
                            in_=qe_ps[:, :fout])

                    kv3 = kvt[:].rearrange("p (c f) -> p c f", c=CH)
                    qk = sb.tile([P, CH * fout], f32, name="qk", tag="qk")
                    nc.vector.tensor_tensor(
                        out=qk[:].rearrange("p (c f) -> p c f", c=CH),
                        in0=qt[:].rearrange("p (c f) -> p c f", c=CH),
                        in1=kv3[:, :, 0:fout],
                        op=mybir.AluOpType.mult,
                    )
                    al = sb.tile([P, CH * HEADS], f32, name="al", tag="al")
                    nc.vector.tensor_reduce(
                        out=al[:].rearrange("p (x o) -> p x o", o=1),
                        in_=qk[:].rearrange("p (x dd) -> p x dd", dd=d),
                        axis=mybir.AxisListType.X,
                        op=mybir.AluOpType.add,
                    )
                    rhs_all = sb.tile([P, CH * FR], bf, name="rhsall", tag="rhsall")
                    r3 = rhs_all[:].rearrange("p (c f) -> p c f", c=CH)
                    nc.scalar.activation(
                        out=r3[:, :, fout:FR],
                        in_=al[:].rearrange("p (c h) -> p c h", c=CH),
                        func=mybir.ActivationFunctionType.Exp,
                        scale=float(1.0 / np.sqrt(d)),
                    )
                    nc.vector.tensor_tensor(
                        out=r3[:, :, 0:fout].rearrange("p c (h dd) -> p c h dd", h=HEADS),
                        in0=r3[:, :, fout:FR][:, :, :, None].to_broadcast(
                            [P, CH, HEADS, d]),
                        in1=kv3[:, :, fout:F2].rearrange(
                            "p c (h dd) -> p c h dd", h=HEADS),
                        op=mybir.AluOpType.mult,
                    )
                    st = sb.tile([P, CH * P], bf, name="st", tag="st")
                    nc.vector.tensor_tensor(
                        out=st[:].rearrange("p (c n) -> p c n", c=CH),
                        in0=dr_s[:, :, None].to_broadcast([P, CH, P]),
                        in1=iota_r[:, None, :].to_broadcast([P, CH, P]),
                        op=mybir.AluOpType.is_equal,
                    )
                    pa = psC.tile([P, FR], f32, name="pa", tag="pa")
                    for c in range(CH):
                        nc.tensor.matmul(
                            pa[:],
                            lhsT=st[:, c * P:(c + 1) * P],
                            rhs=rhs_all[:, c * FR:(c + 1) * FR],
                            start=(c == 0),
                            stop=(c == CH - 1),
                        )
                    den = sb.tile([P, HEADS], f32, name="den", tag="den")
                    nc.vector.tensor_scalar(
                        out=den[:], in0=pa[:, fout:FR], scalar1=1e-16,
                        scalar2=None, op0=mybir.AluOpType.add)
                    rec = sb.tile([P, HEADS], f32, name="rec", tag="rec")
                    nc.vector.reciprocal(out=rec[:], in_=den[:])
                    y = sb.tile([P, fout], f32, name="y", tag="y")
                    nc.vector.tensor_tensor(
                        out=y[:].rearrange("p (h dd) -> p h dd", h=HEADS),
                        in0=pa[:, 0:fout].rearrange("p (h dd) -> p h dd", h=HEADS),
                        in1=rec[:, :, None].to_broadcast([P, HEADS, d]),
                        op=mybir.AluOpType.mult,
                    )
                    yt = psB.tile([P, P], f32, name="yt", tag="pt")
                    nc.tensor.transpose(out=yt[:fout, :], in_=y[:], identity=ident[:])
                    s_s = sb.tile([P, P], f32, name="ss", tag="ss")
                    nc.sync.dma_start(
                        out=s_s[:fout], in_=sT_dram[:, t * P:(t + 1) * P])
                    t1 = sb.tile([P, P], f32, name="t1", tag="t1")
                    nc.vector.tensor_tensor(
                        out=t1[:fout], in0=yt[:fout, :], in1=s_s[:fout],
                        op=mybir.AluOpType.add)
                    if not last:
                        t2 = sb.tile([P, P], f32, name="t2", tag="t2")
                        nc.vector.tensor_scalar(
                            out=t2[:fout], in0=t1[:fout], scalar1=0.0,
                            scalar2=None, op0=mybir.AluOpType.max)
                        nc.vector.tensor_scalar(
                            out=h_next[:, t * P:(t + 1) * P], in0=t2[:fout],
                            scalar1=absb[li][:fout, 0:1],
                            scalar2=absb[li][:fout, 1:2],
                            op0=mybir.AluOpType.mult, op1=mybir.AluOpType.add)
                    else:
                        h4s = sb.tile([P, P], f32, name="h4s", tag="t2")
                        nc.vector.tensor_scalar(
                            out=h4s[:fout], in0=t1[:fout], scalar1=0.0,
                            scalar2=None, op0=mybir.AluOpType.max)
                        nc.sync.dma_start(
                            out=h4T.ap()[:, t * P:(t + 1) * P], in_=h4s[:fout])
                        hbn = sb.tile([P, P], f32, name="hbn", tag="hbn")
                        nc.vector.tensor_scalar(
                            out=hbn[:fout], in0=h4s[:fout],
                            scalar1=absb[li][:fout, 0:1],
                            scalar2=absb[li][:fout, 1:2],
                            op0=mybir.AluOpType.mult, op1=mybir.AluOpType.add)
                        nc.sync.dma_start(
                            out=hbn4T.ap()[:, t * P:(t + 1) * P], in_=hbn[:fout])
                        pl = psB.tile([P, P], f32, name="pl", tag="pt")
                        nc.tensor.matmul(
                            pl[:nclasses, :P], lhsT=wc_sb[:fout],
                            rhs=hbn[:fout], start=True, stop=True)
                        lg = sb.tile([P, P], f32, name="lg", tag="lg")
                        nc.vector.tensor_scalar(
                            out=lg[:nclasses], in0=pl[:nclasses, :P],
                            scalar1=bc_sb[:nclasses, 0:1], scalar2=None,
                            op0=mybir.AluOpType.add)
                        nc.sync.dma_start(
                            out=lgT.ap()[:, t * P:(t + 1) * P],
                            in_=lg[:nclasses])
                h_cur = h_next

    nc.compile()
    return nc


# --------------------------------------------------------------------------
# Public entry point
# --------------------------------------------------------------------------

_CACHE = {}
_LAST_EXEC_NS = None


def kernel_timed(x, edge_index, params):
    """Run kernel with HW profiling; returns max per-core exec_time_ns."""
    global _TRACE
    _TRACE = True
    try:
        kernel(x, edge_index, params)
    finally:
        _TRACE = False
    return _LAST_EXEC_NS


_TRACE = False


def kernel(x, edge_index, params):
    x = np.asarray(x, np.float32)
    edge_index = np.asarray(edge_index)
    n_nodes, gene = x.shape
    src, dst = np.asarray(edge_index[0], np.int64), np.asarray(edge_index[1], np.int64)

    layers = []
    for nm in ("conv1", "conv2", "conv3", "conv4"):
        w = np.asarray(params[nm]["Wq"])
        layers.append((w.shape[0], w.shape[1]))
    nclasses = np.asarray(params["Wc"]).shape[1]

    # tiles per core: smallest T with ~4% headroom for LPT balance
    n_edges = len(src)
    CH = 8
    T = int(np.ceil(n_edges / NCORES * 1.04 / (CH * P)))
    T = max(T, int(np.ceil(n_nodes / (NCORES * P))))
    SLOTS = T * P
    while (SLOTS % 512) != 0:
        T += 1
        SLOTS = T * P

    (slot_global, slot_to_node, src_pc, drel_pc) = _partition_graph(
        n_nodes, src, dst, T, CH)

    key = (n_nodes, gene, tuple(layers), T, CH, nclasses)
    if key not in _CACHE:
        _CACHE[key] = _build_program(gene, layers, T, CH, nclasses)
    nc = _CACHE[key]

    # per-core inputs
    xbf = x.astype(BF16)
    BN_EPS = 1e-5
    wl, bl, abl = [], [], []
    for li, nm in enumerate(("conv1", "conv2", "conv3", "conv4")):
        p = params[nm]
        W = np.stack([np.asarray(p[k], np.float32) for k in ("Wq", "Wk", "Wv", "Ws")], 1)
        fin, _, fout = W.shape
        wl.append(np.ascontiguousarray(W.reshape(fin, 4 * fout)).astype(BF16))
        bl.append(np.ascontiguousarray(np.stack(
            [np.asarray(p[k], np.float32) for k in ("bq", "bk", "bv", "bs")], 1)))
        bn = params[f"bn{li + 1}"]
        A = np.asarray(bn["g"], np.float32) / np.sqrt(np.asarray(bn["v"], np.float32) + BN_EPS)
        B = np.asarray(bn["b"], np.float32) - np.asarray(bn["m"], np.float32) * A
        abl.append(np.ascontiguousarray(np.stack([A, B], 1)))
    wc = np.asarray(params["Wc"], np.float32)
    bc = np.asarray(params["bc"], np.float32).reshape(-1, 1)

    in_maps = []
    for c in range(NCORES):
        s2n = slot_to_node[c]
        valid = s2n >= 0
        xt = np.zeros((gene, SLOTS), BF16)
        xt[:, valid] = xbf[s2n[valid]].T
        m = {
            "xT": xt,
            "srci": src_pc[c], "drel": drel_pc[c],
            "wc": wc, "bc": bc,
        }
        for li in range(4):
            m[f"w{li}"] = wl[li]
            m[f"b{li}"] = bl[li]
            m[f"ab{li}"] = abl[li]
        in_maps.append(m)

    res = run_bass_kernel_spmd(nc, in_maps, core_ids=list(range(NCORES)),
                               trace=_TRACE)
    global _LAST_EXEC_NS
    if _TRACE:
        _LAST_EXEC_NS = res.exec_time_ns

    fout4 = layers[-1][1]
    logits = np.zeros((n_nodes, nclasses), np.float32)
    h4 = np.zeros((n_nodes, fout4), np.float32)
    hbn4 = np.zeros((n_nodes, fout4), np.float32)
    for c in range(NCORES):
        s2n = slot_to_node[c]
        valid = s2n >= 0
        nd = s2n[valid]
        logits[nd] = res.results[c]["lgT"][:, valid].T
        h4[nd] = res.results[c]["h4T"][:, valid].T
        hbn4[nd] = res.results[c]["hbn4T"][:, valid].T
    return logits, h4, hbn4


# revision 13
# speedup vs baseline: 1.4676x; 1.0437x over previous
"""Trainium2 Bass kernel for a 4-layer GraphTransformer (PyG TransformerConv style).

Sharding: nodes are partitioned across 8 NeuronCores (graph parallel) after a
host-side load-balancing relabel; edges are partitioned by destination node so
segment-softmax/scatter are core-local. Per layer each core computes Q/K/V/skip
for its own nodes, K/V (bf16, packed per row) are all-gathered, and the edge
phase gathers per-edge K/V rows (by src) and Q rows (by dst) with indirect DMA.
Segment sums are one-hot matmuls on the TensorEngine accumulating in PSUM.
"""

import sys
import heapq
import numpy as np

sys.path.insert(0, "/opt/trn_rl_repo")

import ml_dtypes
import concourse.bass as bass
import concourse.mybir as mybir
from concourse import bacc, tile
from concourse.bass_utils import run_bass_kernel_spmd
from concourse.masks import make_identity

BF16 = ml_dtypes.bfloat16
NCORES = 8
P = 128
HEADS = 8


# --------------------------------------------------------------------------
# Host-side prep: node->slot assignment (LPT bin packing) and edge arrays
# --------------------------------------------------------------------------

def _partition_graph(n_nodes, src, dst, tiles_per_core, chunks_per_tile):
    """Assign nodes to 8*T tiles of 128 slots, balancing in-degree per tile.

    Returns slot_global[node], slot_to_node[core, slots], per-core edge arrays
    (src_slot, qdst_local, dstrel) shaped [T, 128, CH].
    """
    T = tiles_per_core
    CH = chunks_per_tile
    cap_e = CH * P
    nbins = NCORES * T
    slots_per_core = T * P

    deg = np.bincount(dst, minlength=n_nodes)
    order = np.argsort(-deg, kind="stable")

    bin_load = np.zeros(nbins, np.int64)
    bin_n = np.zeros(nbins, np.int32)
    heap = [(0, b) for b in range(nbins)]
    heapq.heapify(heap)
    node_bin = np.empty(n_nodes, np.int32)
    node_pos = np.empty(n_nodes, np.int32)
    for nd in order:
        d = int(deg[nd])
        while True:
            load, b = heapq.heappop(heap)
            if bin_n[b] < P:
                break
        node_bin[nd] = b
        node_pos[nd] = bin_n[b]
        bin_n[b] += 1
        bin_load[b] = load + d
        if bin_n[b] < P:
            heapq.heappush(heap, (load + d, b))
    if bin_load.max() > cap_e:
        raise RuntimeError(f"bin overflow: {bin_load.max()} > {cap_e}")

    # slot id: bin b covers global slots [b*128, (b+1)*128); core = b // T
    slot_global = node_bin.astype(np.int64) * P + node_pos
    slot_to_node = np.full((NCORES, slots_per_core), -1, np.int64)
    cores = node_bin // T
    local = (node_bin % T) * P + node_pos
    slot_to_node[cores, local] = np.arange(n_nodes)

    # edges sorted by dst slot -> grouped by (core, tile)
    dslot = slot_global[dst]
    order_e = np.argsort(dslot, kind="stable")
    s_sorted = slot_global[src][order_e]
    d_sorted = dslot[order_e]
    ebin = (d_sorted // P).astype(np.int64)
    counts = np.bincount(ebin, minlength=nbins)
    assert counts.max() <= cap_e
    offs = np.zeros(nbins + 1, np.int64)
    np.cumsum(counts, out=offs[1:])
    pos = np.arange(len(d_sorted)) - offs[ebin]

    src_p = np.zeros((nbins, cap_e), np.int32)
    qdst_p = np.zeros((nbins, cap_e), np.int32)
    drel_p = np.full((nbins, cap_e), -1.0, np.float32)
    src_p[ebin, pos] = s_sorted
    qdst_p[ebin, pos] = d_sorted % slots_per_core  # local slot within core
    drel_p[ebin, pos] = d_sorted % P

    def per_core(arr, dt):
        # [nbins, cap_e] -> per core [T, 128, CH]   (edge j: chunk=j//128, p=j%128)
        a = arr.reshape(NCORES, T, CH, P).transpose(0, 1, 3, 2)
        return [np.ascontiguousarray(a[c]).astype(dt) for c in range(NCORES)]

    return (
        slot_global,
        slot_to_node,
        per_core(src_p, np.int32),
        per_core(drel_p, np.float32),
    )


# --------------------------------------------------------------------------
# Device program builder
# --------------------------------------------------------------------------

def _build_program(gene, layers, T, CH, nclasses):
    """layers: list of (fin, fout) for the 4 convs."""
    SLOTS = T * P
    NCH = SLOTS // 512  # node-phase column chunks

    nc = bacc.Bacc("TRN2", target_bir_lowering=False, debug=False,
                   num_devices=NCORES)
    f32, bf, i32 = mybir.dt.float32, mybir.dt.bfloat16, mybir.dt.int32

    xT = nc.dram_tensor("xT", [gene, SLOTS], bf, kind="ExternalInput")
    srcI = nc.dram_tensor("srci", [T, P, CH], i32, kind="ExternalInput")
    drelI = nc.dram_tensor("drel", [T, P, CH], f32, kind="ExternalInput")
    Ws, Bs, ABs = [], [], []
    for li, (fin, fout) in enumerate(layers):
        Ws.append(nc.dram_tensor(f"w{li}", [fin, 4 * fout], bf, kind="ExternalInput"))
        Bs.append(nc.dram_tensor(f"b{li}", [fout, 4], f32, kind="ExternalInput"))
        ABs.append(nc.dram_tensor(f"ab{li}", [fout, 2], f32, kind="ExternalInput"))
    Wc = nc.dram_tensor("wc", [layers[-1][1], nclasses], f32, kind="ExternalInput")
    Bc = nc.dram_tensor("bc", [nclasses, 1], f32, kind="ExternalInput")

    h4T = nc.dram_tensor("h4T", [layers[-1][1], SLOTS], f32, kind="ExternalOutput")
    hbn4T = nc.dram_tensor("hbn4T", [layers[-1][1], SLOTS], f32, kind="ExternalOutput")
    lgT = nc.dram_tensor("lgT", [nclasses, SLOTS], f32, kind="ExternalOutput")

    with tile.TileContext(nc) as tc:
        with (
            tc.tile_pool(name="const", bufs=1) as cpool,
            tc.tile_pool(name="hres", bufs=2) as hpool,
            tc.tile_pool(name="work", bufs=3) as sb,
            tc.tile_pool(name="nwork", bufs=3) as nsb,
            tc.tile_pool(name="psA", bufs=2, space="PSUM") as psA,
            tc.tile_pool(name="psB", bufs=2, space="PSUM") as psB,
            tc.tile_pool(name="psC", bufs=2, space="PSUM") as psC,
            tc.tile_pool(name="dram", bufs=1, space="DRAM") as dr,
        ):
            ident = cpool.tile([P, P], f32, name="ident")
            make_identity(nc, ident[:])
            iota_i = cpool.tile([P, P], i32, name="iota_i")
            nc.gpsimd.iota(iota_i[:], pattern=[[1, P]], base=0, channel_multiplier=0)
            iota_r = cpool.tile([P, P], f32, name="iota_r")  # [p, j] = j
            nc.vector.tensor_copy(out=iota_r[:], in_=iota_i[:])
            iota_ci = cpool.tile([P, P], i32, name="iota_ci")
            nc.gpsimd.iota(iota_ci[:], pattern=[[0, P]], base=0, channel_multiplier=1)
            iota_c = cpool.tile([P, P], f32, name="iota_c")  # [p, j] = p
            nc.vector.tensor_copy(out=iota_c[:], in_=iota_ci[:])

            # per-layer weights resident in SBUF
            wsb, bsb_t, absb = [], [], []
            for li, (fin, fout) in enumerate(layers):
                kt = fin // P
                w = cpool.tile([P, kt * 4 * fout], bf, name=f"wsb{li}")
                for k in range(kt):
                    nc.sync.dma_start(
                        out=w[:, k * 4 * fout:(k + 1) * 4 * fout],
                        in_=Ws[li].ap()[k * P:(k + 1) * P, :],
                    )
                b = cpool.tile([P, 4], f32, name=f"bsb{li}")
                nc.sync.dma_start(out=b[:fout], in_=Bs[li].ap()[:, :])
                ab = cpool.tile([P, 2], f32, name=f"absb{li}")
                nc.sync.dma_start(out=ab[:fout], in_=ABs[li].ap()[:, :])
                wsb.append(w); bsb_t.append(b); absb.append(ab)
            wc_sb = cpool.tile([P, nclasses], f32, name="wcsb")
            nc.sync.dma_start(out=wc_sb[: layers[-1][1]], in_=Wc.ap()[:, :])
            bc_sb = cpool.tile([P, 1], f32, name="bcsb")
            nc.sync.dma_start(out=bc_sb[:nclasses], in_=Bc.ap()[:, :])

            h_cur = None
            for li, (fin, fout) in enumerate(layers):
                kt = fin // P
                d = fout // HEADS
                last = li == len(layers) - 1
                q_dram = dr.tile([SLOTS, fout], bf, name=f"qd{li}")
                kv_core = dr.tile([SLOTS, 2 * fout], bf, name=f"kvc{li}")
                kv_full = dr.tile([NCORES * SLOTS, 2 * fout], bf,
                                  name=f"kvf{li}", addr_space="Shared")
                sT_dram = dr.tile([fout, SLOTS], f32, name=f"sd{li}")
                h_next = None if last else hpool.tile([P, SLOTS], bf, name=f"h{li}",
                                                      tag="hres")

                # ---------------- node phase ----------------
                for ncc in range(NCH):
                    c0 = ncc * 512
                    if li == 0:
                        xt = nsb.tile([P, kt * 512], bf, name="xt")
                        for k in range(kt):
                            nc.sync.dma_start(
                                out=xt[:, k * 512:(k + 1) * 512],
                                in_=xT.ap()[k * P:(k + 1) * P, c0:c0 + 512],
                            )
                        rhs_t = [xt[:, k * 512:(k + 1) * 512] for k in range(kt)]
                    else:
                        rhs_t = [h_cur[:, c0:c0 + 512]]

                    for j in range(4):  # q, k, v, s
                        ps = psA.tile([P, 512], f32, name="psproj", tag="psproj")
                        for k in range(kt):
                            nc.tensor.matmul(
                                ps[:fout],
                                lhsT=wsb[li][:, (k * 4 + j) * fout:(k * 4 + j + 1) * fout],
                                rhs=rhs_t[k],
                                start=(k == 0),
                                stop=(k == kt - 1),
                            )
                        fs = nsb.tile([P, 512], f32, name="fs", tag="fs")
                        nc.vector.tensor_scalar(
                            out=fs[:fout], in0=ps[:fout],
                            scalar1=bsb_t[li][:fout, j:j + 1], scalar2=None,
                            op0=mybir.AluOpType.add,
                        )
                        if j == 3:  # skip proj: store transposed f32
                            nc.sync.dma_start(
                                out=sT_dram[:, c0:c0 + 512], in_=fs[:fout])
                            continue
                        bb = nsb.tile([P, 4 * fout], bf, name="bb", tag=f"bb{j}")
                        for i in range(4):
                            pt = psB.tile([P, P], f32, name="pt", tag="pt")
                            nc.tensor.transpose(
                                out=pt[:, :fout],
                                in_=fs[:fout, i * P:(i + 1) * P],
                                identity=ident[:fout, :fout],
                            )
                            nc.scalar.copy(
                                out=bb[:, i * fout:(i + 1) * fout],
                                in_=pt[:, :fout])
                        if j == 0:
                            oap = q_dram[c0:c0 + 512, :]
                        elif j == 1:
                            oap = kv_core[c0:c0 + 512, 0:fout]
                        else:
                            oap = kv_core[c0:c0 + 512, fout:2 * fout]
                        nc.sync.dma_start(
                            out=oap.rearrange("(i p) f -> p i f", p=P),
                            in_=bb[:].rearrange("p (i f) -> p i f", i=4),
                        )

                # ---------------- all-gather K/V ----------------
                nc.gpsimd.collective_compute(
                    "AllGather", mybir.AluOpType.bypass,
                    replica_groups=[list(range(NCORES))],
                    ins=[kv_core.opt()], outs=[kv_full.opt()],
                )

                # ---------------- edge phase ----------------
                F2 = 2 * fout
                FR = fout + HEADS  # rhs cols per chunk: [wv | exp]
                for t in range(T):
                    src_s = sb.tile([P, CH], i32, name="srcs", tag="srcs")
                    nc.sync.dma_start(out=src_s[:], in_=srcI.ap()[t])
                    dr_s = sb.tile([P, CH], f32, name="drs", tag="drs")
                    nc.sync.dma_start(out=dr_s[:], in_=drelI.ap()[t])
                    q_t = sb.tile([P, fout], bf, name="q_t", tag="q_t")
                    nc.sync.dma_start(
                        out=q_t[:], in_=q_dram[t * P:(t + 1) * P, :])

                    kvt = sb.tile([P, CH * F2], bf, name="kvt", tag="kvt")
                    for c in range(CH):
                        nc.gpsimd.indirect_dma_start(
                            out=kvt[:, c * F2:(c + 1) * F2], out_offset=None,
                            in_=kv_full[:],
                            in_offset=bass.IndirectOffsetOnAxis(
                                ap=src_s[:, c:c + 1], axis=0),
                        )

                    # expand q to edges: per chunk build S[n, e] one-hot and
                    # matmul with the dense q tile of this dst window
                    qt = sb.tile([P, CH * fout], bf, name="qt", tag="qt")
                    for c in range(CH):
                        ptt = psB.tile([P, P], f32, name="ptt", tag="ptt")
                        nc.tensor.transpose(
                            out=ptt[:],
                            in_=dr_s[:, c:c + 1].to_broadcast([P, P]),
                            identity=ident[:])
                        S = sb.tile([P, P], bf, name="S", tag="S")
                        nc.vector.tensor_tensor(
                            out=S[:], in0=ptt[:], in1=iota_c[:],
                            op=mybir.AluOpType.is_equal)
                        qe_ps = psB.tile([P, P], f32, name="qe_ps", tag="pt")
                        nc.tensor.matmul(
                            qe_ps[:, :fout], lhsT=S[:], rhs=q_t[:],
                            start=True, stop=True)
                        nc.scalar.copy(
                            out=qt[:, c * fout:(c + 1) * fout],
                            in_=qe_ps[:, :fout])

                    kv3 = kvt[:].rearrange("p (c f) -> p c f", c=CH)
                    qk = sb.tile([P, CH * fout], f32, name="qk", tag="qk")
                    nc.vector.tensor_tensor(
                        out=qk[:].rearrange("p (c f) -> p c f", c=CH),
                        in0=qt[:].rearrange("p (c f) -> p c f", c=CH),
                        in1=kv3[:, :, 0:fout],
                        op=mybir.AluOpType.mult,
                    )
                    al = sb.tile([P, CH * HEADS], f32, name="al", tag="al")
                    nc.vector.tensor_reduce(
                        out=al[:].rearrange("p (x o) -> p x o", o=1),
                        in_=qk[:].rearrange("p (x dd) -> p x dd", dd=d),
                        axis=mybir.AxisListType.X,
                        op=mybir.AluOpType.add,
                    )
                    rhs_all = sb.tile([P, CH * FR], bf, name="rhsall", tag="rhsall")
                    r3 = rhs_all[:].rearrange("p (c f) -> p c f", c=CH)
                    nc.scalar.activation(
                        out=r3[:, :, fout:FR],
                        in_=al[:].rearrange("p (c h) -> p c h", c=CH),
                        func=mybir.ActivationFunctionType.Exp,
                        scale=float(1.0 / np.sqrt(d)),
                    )
                    nc.vector.tensor_tensor(
                        out=r3[:, :, 0:fout].rearrange("p c (h dd) -> p c h dd", h=HEADS),
                        in0=r3[:, :, fout:FR][:, :, :, None].to_broadcast(
                            [P, CH, HEADS, d]),
                        in1=kv3[:, :, fout:F2].rearrange(
                            "p c (h dd) -> p c h dd", h=HEADS),
                        op=mybir.AluOpType.mult,
                    )
                    st = sb.tile([P, CH * P], bf, name="st", tag="st")
                    nc.vector.tensor_tensor(
                        out=st[:].rearrange("p (c n) -> p c n", c=CH),
                        in0=dr_s[:, :, None].to_broadcast([P, CH, P]),
                        in1=iota_r[:, None, :].to_broadcast([P, CH, P]),
                        op=mybir.AluOpType.is_equal,
                    )
                    pa = psC.tile([P, FR], f32, name="pa", tag="pa")
                    for c in range(CH):
                        nc.tensor.matmul(
                            pa[:],
                            lhsT=st[:, c * P:(c + 1) * P],
                            rhs=rhs_all[:, c * FR:(c + 1) * FR],
                            start=(c == 0),
                            stop=(c == CH - 1),
                        )
                    den = sb.tile([P, HEADS], f32, name="den", tag="den")
                    nc.vector.tensor_scalar(
                        out=den[:], in0=pa[:, fout:FR], scalar1=1e-16,
                        scalar2=None, op0=mybir.AluOpType.add)
                    rec = sb.tile([P, HEADS], f32, name="rec", tag="rec")
                    nc.vector.reciprocal(out=rec[:], in_=den[:])
                    y = sb.tile([P, fout], f32, name="y", tag="y")
                    nc.vector.tensor_tensor(
                        out=y[:].rearrange("p (h dd) -> p h dd", h=HEADS),
                        in0=pa[:, 0:fout].rearrange("p (h dd) -> p h dd", h=HEADS),
                        in1=rec[:, :, None].to_broadcast([P, HEADS, d]),
                        op=mybir.AluOpType.mult,
                    )
                    yt = psB.tile([P, P], f32, name="yt", tag="pt")
                    nc.tensor.transpose(out=yt[:fout, :], in_=y[:], identity=ident[:])
                    s_s = sb.tile([P, P], f32, name="ss", tag="ss")
                    nc.sync.dma_start(
                        out=s_s[:fout], in_=sT_dram[:, t * P:(t + 1) * P])
                    t1 = sb.tile([P, P], f32, name="t1", tag="t1")
                    nc.vector.tensor_tensor(
                        out=t1[:fout], in0=yt[:fout, :], in1=s_s[:fout],
                        op=mybir.AluOpType.add)
                    if not last:
                        t2 = sb.tile([P, P], f32, name="t2", tag="t2")
                        nc.vector.tensor_scalar(
                            out=t2[:fout], in0=t1[:fout], scalar1=0.0,
                            scalar2=None, op0=mybir.AluOpType.max)
                        nc.vector.tensor_scalar(
                            out=h_next[:, t * P:(t + 1) * P], in0=t2[:fout],
                            scalar1=absb[li][:fout, 0:1],
                            scalar2=absb[li][:fout, 1:2],
                            op0=mybir.AluOpType.mult, op1=mybir.AluOpType.add)
                    else:
                        h4s = sb.tile([P, P], f32, name="h4s", tag="t2")
                        nc.vector.tensor_scalar(
                            out=h4s[:fout], in0=t1[:fout], scalar1=0.0,
                            scalar2=None, op0=mybir.AluOpType.max)
                        nc.sync.dma_start(
                            out=h4T.ap()[:, t * P:(t + 1) * P], in_=h4s[:fout])
                        hbn = sb.tile([P, P], f32, name="hbn", tag="hbn")
                        nc.vector.tensor_scalar(
                            out=hbn[:fout], in0=h4s[:fout],
                            scalar1=absb[li][:fout, 0:1],
                            scalar2=absb[li][:fout, 1:2],
                            op0=mybir.AluOpType.mult, op1=mybir.AluOpType.add)
                        nc.sync.dma_start(
                            out=hbn4T.ap()[:, t * P:(t + 1) * P], in_=hbn[:fout])
                        pl = psB.tile([P, P], f32, name="pl", tag="pt")
                        nc.tensor.matmul(
                            pl[:nclasses, :P], lhsT=wc_sb[:fout],
                            rhs=hbn[:fout], start=True, stop=True)
                        lg = sb.tile([P, P], f32, name="lg", tag="lg")
                        nc.vector.tensor_scalar(
                            out=lg[:nclasses], in0=pl[:nclasses, :P],
                            scalar1=bc_sb[:nclasses, 0:1], scalar2=None,
                            op0=mybir.AluOpType.add)
                        nc.sync.dma_start(
                            out=lgT.ap()[:, t * P:(t + 1) * P],
                            in_=lg[:nclasses])
                h_cur = h_next

    nc.compile()
    return nc


# --------------------------------------------------------------------------
# Public entry point
# --------------------------------------------------------------------------

_CACHE = {}
_LAST_EXEC_NS = None


def kernel_timed(x, edge_index, params):
    """Run kernel with HW profiling; returns max per-core exec_time_ns."""
    global _TRACE
    _TRACE = True
    try:
        kernel(x, edge_index, params)
    finally:
        _TRACE = False
    return _LAST_EXEC_NS


_TRACE = False


def kernel(x, edge_index, params):
    x = np.asarray(x, np.float32)
    edge_index = np.asarray(edge_index)
    n_nodes, gene = x.shape
    src, dst = np.asarray(edge_index[0], np.int64), np.asarray(edge_index[1], np.int64)

    layers = []
    for nm in ("conv1", "conv2", "conv3", "conv4"):
        w = np.asarray(params[nm]["Wq"])
        layers.append((w.shape[0], w.shape[1]))
    nclasses = np.asarray(params["Wc"]).shape[1]

    # tiles per core: smallest T with ~4% headroom for LPT balance
    n_edges = len(src)
    CH = 8
    T = int(np.ceil(n_edges / NCORES * 1.04 / (CH * P)))
    T = max(T, int(np.ceil(n_nodes / (NCORES * P))))
    SLOTS = T * P
    while (SLOTS % 512) != 0:
        T += 1
        SLOTS = T * P

    (slot_global, slot_to_node, src_pc, drel_pc) = _partition_graph(
        n_nodes, src, dst, T, CH)

    key = (n_nodes, gene, tuple(layers), T, CH, nclasses)
    if key not in _CACHE:
        _CACHE[key] = _build_program(gene, layers, T, CH, nclasses)
    nc = _CACHE[key]

    # per-core inputs
    xbf = x.astype(BF16)
    BN_EPS = 1e-5
    wl, bl, abl = [], [], []
    for li, nm in enumerate(("conv1", "conv2", "conv3", "conv4")):
        p = params[nm]
        W = np.stack([np.asarray(p[k], np.float32) for k in ("Wq", "Wk", "Wv", "Ws")], 1)
        fin, _, fout = W.shape
        wl.append(np.ascontiguousarray(W.reshape(fin, 4 * fout)).astype(BF16))
        bl.append(np.ascontiguousarray(np.stack(
            [np.asarray(p[k], np.float32) for k in ("bq", "bk", "bv", "bs")], 1)))
        bn = params[f"bn{li + 1}"]
        A = np.asarray(bn["g"], np.float32) / np.sqrt(np.asarray(bn["v"], np.float32) + BN_EPS)
        B = np.asarray(bn["b"], np.float32) - np.asarray(bn["m"], np.float32) * A
        abl.append(np.ascontiguousarray(np.stack([A, B], 1)))
    wc = np.asarray(params["Wc"], np.float32)
    bc = np.asarray(params["bc"], np.float32).reshape(-1, 1)

    in_maps = []
    for c in range(NCORES):
        s2n = slot_to_node[c]
        valid = s2n >= 0
        xt = np.zeros((gene, SLOTS), BF16)
        xt[:, valid] = xbf[s2n[valid]].T
        m = {
            "xT": xt,
            "srci": src_pc[c], "drel": drel_pc[c],
            "wc": wc, "bc": bc,
        }
        for li in range(4):
            m[f"w{li}"] = wl[li]
            m[f"b{li}"] = bl[li]
            m[f"ab{li}"] = abl[li]
        in_maps.append(m)

    res = run_bass_kernel_spmd(nc, in_maps, core_ids=list(range(NCORES)),
                               trace=_TRACE)
    global _LAST_EXEC_NS
    if _TRACE:
        _LAST_EXEC_NS = res.exec_time_ns

    fout4 = layers[-1][1]
    logits = np.zeros((n_nodes, nclasses), np.float32)
    h4 = np.zeros((n_nodes, fout4), np.float32)
    hbn4 = np.zeros((n_nodes, fout4), np.float32)
    for c in range(NCORES):
        s2n = slot_to_node[c]
        valid = s2n >= 0
        nd = s2n[valid]
        logits[nd] = res.results[c]["lgT"][:, valid].T
        h4[nd] = res.results[c]["h4T"][:, valid].T
        hbn4[nd] = res.results[c]["hbn4T"][:, valid].T
    return logits, h4, hbn4
